# revision 1
# baseline (speedup 1.0000x reference)
"""Weighted-BCE loss kernel for Trainium2 (8 NeuronCores, SPMD data-parallel).

Reference math (torch-style BCELoss with class-balancing weights):
    n   = len(x), s = sum(gt)
    w0  = n / (2*(n-s)),  w1 = n / (2*s)
    L1  = max(log(x),     -100)
    L0  = max(log1p(-x),  -100)
    loss = mean( where(gt==0, w0, w1) * -(gt*L1 + (1-gt)*L0) )

The weights depend only on the GLOBAL positive count s, so the loss
decomposes into 4 global sums computed shard-locally:
    A = sum(gt * L1),  B = sum(gt * L0),  C = sum(L0),  s = sum(gt)
    loss = -( A/(2s) + (C-B)/(2(n-s)) )

Each core processes a 1/8 shard laid out [128 partitions, 16384 free]:
  - ScalarE (ACT): Ln(x), and Ln(1-x) via the free affine (scale=-1,
    bias=1); the second op's accum_out produces C for free; a Copy
    activation of gt with accum_out produces s.  ACT also issues the gt
    DMAs so x and gt stream through two separate HWDGE queues.
  - VectorE (DVE): two fused scalar_tensor_tensor ops, each doing
    clamp(max, -100) + multiply-by-gt + row-reduce in one instruction
    (A and B).  gt (int32) is consumed directly as the in1 operand.
All engines stay near the DMA roofline (16.8 MB/core @ 358 GB/s ~ 47us).
Host gathers the [128, 4*ntiles] partials from all 8 cores and finishes
the (tiny) all-reduce + final scalar arithmetic in float64.
"""

import numpy as np
from contextlib import ExitStack

import concourse.bass as bass
import concourse.bacc as bacc
import concourse.mybir as mybir
import concourse.tile as tile
from concourse.alu_op_type import AluOpType
from concourse.bass_utils import run_bass_kernel_spmd

N_TOTAL = 16777216
N_CORES = 8
PER_CORE = N_TOTAL // N_CORES   # 2097152
P = 128
FD = PER_CORE // P              # 16384 free elements per partition
# uniform large tiles measured fastest: per-instruction + semaphore overhead
# of extra small tiles outweighs the ramp/tail savings they buy
TILE_SIZES = [4096, 4096, 4096, 4096]
assert sum(TILE_SIZES) == FD
NT = len(TILE_SIZES)
# s-sum runs on ACT (copy+accum) for every tile; all DVE-side s variants
# (including tile-0-only, where DVE idles during ramp) measured 4-10us slower
S_ON_ACT = {0, 1, 2, 3}
LOG_CLAMP = -100.0

# Optional instrumentation knobs for a driver script (harness never sets them).
TRACE = False
LAST_RESULTS = None

_NC_CACHE = None


def _build():
    f32 = mybir.dt.float32
    i32 = mybir.dt.int32
    Ln = mybir.ActivationFunctionType.Ln

    nc = bacc.Bacc("TRN2")
    x_in = nc.declare_dram_parameter("x", [P, FD], f32, isOutput=False)
    g_in = nc.declare_dram_parameter("gt", [P, FD], i32, isOutput=False)
    # one packed output: columns [A | B | C | S], NT each
    out_all = nc.declare_dram_parameter("out_all", [P, 4 * NT], f32, isOutput=True)

    with tile.TileContext(nc) as tc, ExitStack() as ctx:
        xp = ctx.enter_context(tc.tile_pool(name="xp", bufs=2))
        gp = ctx.enter_context(tc.tile_pool(name="gp", bufs=3))
        lp = ctx.enter_context(tc.tile_pool(name="lp", bufs=2))
        jp = ctx.enter_context(tc.tile_pool(name="jp", bufs=1))
        accp = ctx.enter_context(tc.tile_pool(name="accp", bufs=1))

        accA = accp.tile([P, NT], f32)
        accB = accp.tile([P, NT], f32)
        accC = accp.tile([P, NT], f32)
        accS = accp.tile([P, NT], f32)
        groups = [accA, accB, accC, accS]

        def col(group, i):
            return groups[group][:, i : i + 1]

        off = 0
        for i, tfd in enumerate(TILE_SIZES):
            sl = slice(off, off + tfd)
            off += tfd
            xt = xp.tile([P, tfd], f32, tag="xt")
            gt_t = gp.tile([P, tfd], i32, tag="gt")
            # two HWDGE queues: x via SP(sync), gt via the ACT sequencer
            nc.sync.dma_start(xt[:], x_in[:, sl])
            nc.scalar.dma_start(gt_t[:], g_in[:, sl])

            lnx = lp.tile([P, tfd], f32, tag="lnx")
            ln1 = lp.tile([P, tfd], f32, tag="ln1")
            nc.scalar.activation(lnx[:], xt[:], Ln)
            nc.scalar.activation(
                ln1[:], xt[:], Ln, bias=1.0, scale=-1.0,
                accum_out=col(2, i),
            )

            junk = jp.tile([P, tfd], f32, tag="junk")
            nc.vector.scalar_tensor_tensor(
                junk[:], lnx[:], LOG_CLAMP, gt_t[:],
                AluOpType.max, AluOpType.mult,
                accum_out=col(0, i),
            )
            junk2 = jp.tile([P, tfd], f32, tag="junk")
            nc.vector.scalar_tensor_tensor(
                junk2[:], ln1[:], LOG_CLAMP, gt_t[:],
                AluOpType.max, AluOpType.mult,
                accum_out=col(1, i),
            )
            # s = sum(gt), load-balanced between ACT (copy+accum) and DVE
            # (STT: (junk*0) + gt with accum; junk is finite by construction)
            junk3 = jp.tile([P, tfd], f32, tag="junk3")
            if i in S_ON_ACT:
                nc.scalar.activation(
                    junk3[:], gt_t[:], mybir.ActivationFunctionType.Copy,
                    accum_out=col(3, i),
                )
            else:
                nc.vector.scalar_tensor_tensor(
                    junk3[:], junk[:], 0.0, gt_t[:],
                    AluOpType.mult, AluOpType.add,
                    accum_out=col(3, i),
                )

        for k, g in enumerate(groups):
            nc.sync.dma_start(out_all[:, k * NT : (k + 1) * NT], g[:])

    nc.compile()
    return nc


def get_nc():
    global _NC_CACHE
    if _NC_CACHE is None:
        _NC_CACHE = _build()
    return _NC_CACHE


def make_in_maps(x, gt):
    x = np.ascontiguousarray(np.asarray(x, dtype=np.float32).reshape(-1))
    gt = np.ascontiguousarray(np.asarray(gt, dtype=np.int32).reshape(-1))
    assert x.shape == (N_TOTAL,) and gt.shape == (N_TOTAL,)
    in_maps = []
    for c in range(N_CORES):
        sl = slice(c * PER_CORE, (c + 1) * PER_CORE)
        in_maps.append({
            "x": x[sl].reshape(P, FD),
            "gt": gt[sl].reshape(P, FD),
        })
    return in_maps


def combine(results):
    """All-reduce the per-core partial sums and finish the loss formula."""
    A = B = C = S = 0.0
    for r in results:
        o = r["out_all"].astype(np.float64)
        A += o[:, 0 * NT : 1 * NT].sum()
        B += o[:, 1 * NT : 2 * NT].sum()
        C += o[:, 2 * NT : 3 * NT].sum()
        S += o[:, 3 * NT : 4 * NT].sum()
    n = float(N_TOTAL)
    result = -(A / (2.0 * S) + (C - B) / (2.0 * (n - S)))
    return np.array(result, dtype=np.float32)


def kernel(x, gt):
    global LAST_RESULTS
    nc = get_nc()
    in_maps = make_in_maps(x, gt)
    br = run_bass_kernel_spmd(nc, in_maps, list(range(N_CORES)))
    LAST_RESULTS = br
    return combine(br.results)



# revision 5
# speedup vs baseline: 1.2005x; 1.2005x over previous
"""Weighted-BCE loss kernel for Trainium2 (8 NeuronCores, SPMD data-parallel).

Reference math (torch-style BCELoss with class-balancing weights):
    n   = len(x), s = sum(gt)
    w0  = n / (2*(n-s)),  w1 = n / (2*s)
    L1  = max(log(x),     -100)
    L0  = max(log1p(-x),  -100)
    loss = mean( where(gt==0, w0, w1) * -(gt*L1 + (1-gt)*L0) )

Only ONE of log(x) / log(1-x) is needed per element (selected by gt), so
instead of two Ln passes we compute the selected operand in one shot:
    z = gt ? x : 1-x  =  1 - |x - gt|          (gt in {0,1})
With x' = max(x, 2^-24) (free: folded into the op0 slot of the STT that
forms w = x' - gt) we get z >= 2^-24, so Ln(z) >= -16.64 and no -inf can
arise; the reference's -100 clamp only differs for exact x==0 elements
(expected ~1 in 16.7M, error ~5e-6 of the loss - far below tolerance).

Global sums computed shard-locally (weights only need the GLOBAL s):
    A = sum(gt * Lz)   [DVE STT accum]  = sum_{gt=1} log x
    T = sum(Lz)        [ACT accum, free on the Ln pass]
    S = sum(gt)        [ACT Copy accum]
    loss = -( A/(2S) + (T-A)/(2(n-S)) )

Engine balance per [128, tfd] tile (DMA needs ~12us/4096-tile, all
engines must stay under that).  GpSimd/Pool cannot run TensorScalarPtr
on core v3 (codegen rejects it), so the |w| op alternates between ACT
(Abs activation - same natural_log table set as Ln/Copy, no table
swaps) and DVE (tensor_scalar abs_max):
    DVE    w-STT + A-STT (+abs on even tiles)     avg ~11.4us
    ACT    Ln(1-d) w/ accum + Copy(gt) w/ accum
           (+Abs on odd tiles) + gt DMA           avg ~10.1us
    SP     x DMA
Descending tile sizes shrink the pipeline-drain tail (the last tile's
w->d->Ln->A chain runs after the final DMA lands).
Host gathers [128, 3*NT] partials from all 8 cores and finishes the
(tiny) all-reduce + final scalar arithmetic in float64.
"""

import numpy as np
from contextlib import ExitStack

import concourse.bass as bass
import concourse.bacc as bacc
import concourse.mybir as mybir
import concourse.tile as tile
from concourse.alu_op_type import AluOpType
from concourse.bass_utils import run_bass_kernel_spmd

N_TOTAL = 16777216
N_CORES = 8
PER_CORE = N_TOTAL // N_CORES   # 2097152
P = 128
FD = PER_CORE // P              # 16384 free elements per partition
TILE_SIZES = [4096, 4096, 4096, 2048, 1024, 512, 512]
assert sum(TILE_SIZES) == FD
NT = len(TILE_SIZES)
X_CLAMP = 5.9604645e-08         # 2^-24: keeps z = 1-|x'-gt| >= 2^-24
LOG_CLAMP = -100.0

# Optional instrumentation knobs for a driver script (harness never sets them).
TRACE = False
LAST_RESULTS = None

_NC_CACHE = None


def _build():
    f32 = mybir.dt.float32
    i32 = mybir.dt.int32
    Ln = mybir.ActivationFunctionType.Ln

    nc = bacc.Bacc("TRN2")
    x_in = nc.declare_dram_parameter("x", [P, FD], f32, isOutput=False)
    g_in = nc.declare_dram_parameter("gt", [P, FD], i32, isOutput=False)
    # one packed output: columns [A | T | S], NT each
    out_all = nc.declare_dram_parameter("out_all", [P, 3 * NT], f32, isOutput=True)

    with tile.TileContext(nc) as tc, ExitStack() as ctx:
        xp = ctx.enter_context(tc.tile_pool(name="xp", bufs=2))
        gp = ctx.enter_context(tc.tile_pool(name="gp", bufs=2))
        wp = ctx.enter_context(tc.tile_pool(name="wp", bufs=2))
        dp = ctx.enter_context(tc.tile_pool(name="dp", bufs=2))
        lp = ctx.enter_context(tc.tile_pool(name="lp", bufs=2))
        jp = ctx.enter_context(tc.tile_pool(name="jp", bufs=1))
        jsp = ctx.enter_context(tc.tile_pool(name="jsp", bufs=1))
        accp = ctx.enter_context(tc.tile_pool(name="accp", bufs=1))

        accA = accp.tile([P, NT], f32)
        accT = accp.tile([P, NT], f32)
        accS = accp.tile([P, NT], f32)
        groups = [accA, accT, accS]

        def col(group, i):
            return groups[group][:, i : i + 1]

        off = 0
        for i, tfd in enumerate(TILE_SIZES):
            sl = slice(off, off + tfd)
            off += tfd
            xt = xp.tile([P, tfd], f32, tag="xt")
            gt_t = gp.tile([P, tfd], i32, tag="gt")
            # two HWDGE queues: x via SP(sync), gt via the ACT sequencer
            nc.sync.dma_start(xt[:], x_in[:, sl])
            nc.scalar.dma_start(gt_t[:], g_in[:, sl])

            # w = max(x, 2^-24) - gt  in {(0,1), (-1,0)};  |w| <= 1 - 2^-24
            wt = wp.tile([P, tfd], f32, tag="w")
            nc.vector.scalar_tensor_tensor(
                wt[:], xt[:], X_CLAMP, gt_t[:],
                AluOpType.max, AluOpType.subtract,
            )
            # d = |w|, alternating engines to balance ACT vs DVE load
            dt_ = dp.tile([P, tfd], f32, tag="d")
            if i % 2 == 0:
                # |w| = max(-w, w) via STT (abs_max is not a valid TS op)
                nc.vector.scalar_tensor_tensor(
                    dt_[:], wt[:], -1.0, wt[:],
                    AluOpType.mult, AluOpType.max,
                )
            else:
                nc.scalar.activation(
                    dt_[:], wt[:], mybir.ActivationFunctionType.Abs
                )

            # Lz = Ln(1 - d) = log(gt ? x' : 1-x'), accum -> T
            lz = lp.tile([P, tfd], f32, tag="lz")
            nc.scalar.activation(
                lz[:], dt_[:], Ln, bias=1.0, scale=-1.0,
                accum_out=col(1, i),
            )
            # S = sum(gt) via ACT Copy (i32 -> f32) accum
            junk_s = jsp.tile([P, tfd], f32, tag="junk_s")
            nc.scalar.activation(
                junk_s[:], gt_t[:], mybir.ActivationFunctionType.Copy,
                accum_out=col(2, i),
            )
            # A = sum(gt * Lz); the max(-100) clamp is a no-op safety net
            junk_a = jp.tile([P, tfd], f32, tag="junk_a")
            nc.vector.scalar_tensor_tensor(
                junk_a[:], lz[:], LOG_CLAMP, gt_t[:],
                AluOpType.max, AluOpType.mult,
                accum_out=col(0, i),
            )

        for k, g in enumerate(groups):
            nc.sync.dma_start(out_all[:, k * NT : (k + 1) * NT], g[:])

    nc.compile()
    return nc


def get_nc():
    global _NC_CACHE
    if _NC_CACHE is None:
        _NC_CACHE = _build()
    return _NC_CACHE


def make_in_maps(x, gt):
    x = np.ascontiguousarray(np.asarray(x, dtype=np.float32).reshape(-1))
    gt = np.ascontiguousarray(np.asarray(gt, dtype=np.int32).reshape(-1))
    assert x.shape == (N_TOTAL,) and gt.shape == (N_TOTAL,)
    in_maps = []
    for c in range(N_CORES):
        sl = slice(c * PER_CORE, (c + 1) * PER_CORE)
        in_maps.append({
            "x": x[sl].reshape(P, FD),
            "gt": gt[sl].reshape(P, FD),
        })
    return in_maps


def combine(results):
    """All-reduce the per-core partial sums and finish the loss formula."""
    A = T = S = 0.0
    for r in results:
        o = r["out_all"].astype(np.float64)
        A += o[:, 0 * NT : 1 * NT].sum()
        T += o[:, 1 * NT : 2 * NT].sum()
        S += o[:, 2 * NT : 3 * NT].sum()
    n = float(N_TOTAL)
    result = -(A / (2.0 * S) + (T - A) / (2.0 * (n - S)))
    return np.array(result, dtype=np.float32)


def kernel(x, gt):
    global LAST_RESULTS
    nc = get_nc()
    in_maps = make_in_maps(x, gt)
    br = run_bass_kernel_spmd(nc, in_maps, list(range(N_CORES)))
    LAST_RESULTS = br
    return combine(br.results)
